# revision 1
# baseline (speedup 1.0000x reference)
"""Trainium2 Bass kernel for nn_CLUBCategorical (CLUB categorical loss).

Reference computation:
    h      = relu(x @ W1 + b1)              [N, H]
    logits = h @ W2 + b2                    [N, Y]
    logp   = log_softmax(logits, -1)        [N, Y]
    out[i] = logp[i, y_i] - mean_j logp[i, y_j]

Algebraic simplification: with c[y] = histogram(y_idx), the log-softmax
normalizer cancels between the positive and negative terms:

    out[i] = L[i, y_i] - (1/N) * (L[i, :] @ c) + (b2[y_i] - (b2 @ c)/N)

where L = relu(x @ W1 + b1) @ W2 (no bias, no softmax). On device this is
two dense matmuls plus a masked column reduction:

    out[i] = sum_y L[i, y] * (onehot(y_i)[y] - c[y]/N) + g[i]

Sharding: data-parallel over N. Each of the 8 cores handles 1024 rows and
gets the full W1/W2 plus the global label histogram (the "all-gather of
column labels" is folded into c on the host). No collectives needed.

Device layout (per core; contraction dim always on SBUF partitions, all
operand layouts pre-arranged on host so every DMA is one contiguous
descriptor):
    phase 1: hT[m]  [128h, 1024r] = W1[:,mslice].T @ xT[:, rows] (+b1, relu)
    phase 2: psum_l [128y,  512r] = W2[:,qslice].T @ hT[:, rows]
             eqc    [128y,  512r] = (ybc == iota_q) - cN_q       (DVE)
             prod   = psum_l * eqc                               (DVE)
             out    += ones.T @ prod  (M=1 matmul reduces over y) (PE)
ybc is broadcast on device from a [1, rows] vector via a K=1 matmul.
Matmuls run in float32r (~2^-13 relative precision, 2x fp32 throughput).
DMA descriptors are interleaved across the two HWDGE queues (sync,
scalar) in phase-1 consumption order; constants ride the gpsimd SWDGE.
"""

import numpy as np

N, X_DIM, Y_DIM, HIDDEN = 8192, 512, 512, 1024
N_CORES = 8
N_LOC = N // N_CORES          # 1024 rows per core
KX = X_DIM // 128             # 4  k-chunks, phase 1
KH = HIDDEN // 128            # 8  k-chunks, phase 2 / m-chunks, phase 1
QY = Y_DIM // 128             # 4  y-chunks, phase 2
RG = N_LOC // 512             # 2  row groups of 512

_NC_CACHE = {}


def _build(nc_cls, mybir, tile):
    mdt = mybir.dt
    f32 = mdt.float32
    F32R = mdt.float32r
    AF = mybir.ActivationFunctionType
    OP = mybir.AluOpType

    nc = nc_cls("TRN2", target_bir_lowering=False, debug=False,
                num_devices=N_CORES)

    # xt{n}{a,b}: x rows for row-group n, partition-major, k-halves
    xtD = [[nc.dram_tensor(f"xt{n}{h}", [128, 2 * 512], f32,
                           kind="ExternalInput") for h in "ab"]
           for n in range(RG)]
    # w1p{mp}: W1 columns for hidden-pair mp, all K
    w1D = [nc.dram_tensor(f"w1p{mp}", [128, KX * 256], f32,
                          kind="ExternalInput") for mp in range(KH // 2)]
    # w2p{h}: W2 rows half h, partition-major
    w2D = [nc.dram_tensor(f"w2p{h}", [128, 4 * Y_DIM], f32,
                          kind="ExternalInput") for h in range(2)]
    # packed constants: [b1c(8) | iot(4) | cNc(4) | ones(1)] = [128, 17]
    cst = nc.dram_tensor("cst", [128, KH + 2 * QY + 1], f32,
                         kind="ExternalInput")
    o128 = nc.dram_tensor("o128", [1, 128], f32, kind="ExternalInput")
    yrow = nc.dram_tensor("yrow", [1, N_LOC], f32, kind="ExternalInput")
    gv = nc.dram_tensor("gv", [1, N_LOC], f32, kind="ExternalInput")
    out = nc.dram_tensor("out", [1, N_LOC], f32, kind="ExternalOutput")

    with tile.TileContext(nc) as tc:
        with (
            tc.tile_pool(name="wgt", bufs=1) as wgt,
            tc.tile_pool(name="hp", bufs=1) as hp,
            tc.tile_pool(name="eqp", bufs=1) as eqp,
            tc.tile_pool(name="prp", bufs=4) as prp,
            tc.tile_pool(name="osb", bufs=1) as osb,
            tc.tile_pool(name="ps", bufs=1, space="PSUM") as ps,
        ):
            cst_sb = wgt.tile([128, KH + 2 * QY + 1], F32R, tag="cst")
            b1_sb = cst_sb[:, 0:KH].bitcast(f32)
            iot_sb = cst_sb[:, KH:KH + QY].bitcast(f32)
            cnc_sb = cst_sb[:, KH + QY:KH + 2 * QY].bitcast(f32)
            ones_sb = cst_sb[:, KH + 2 * QY:KH + 2 * QY + 1]
            yrow_sb = wgt.tile([1, N_LOC], F32R, tag="yrow")
            o128_sb = wgt.tile([1, 128], F32R, tag="o128")
            g_sb = wgt.tile([1, N_LOC], f32, tag="g")
            nc.gpsimd.dma_start(g_sb[:], gv.ap())

            # --- big loads, interleaved across both HWDGE queues in
            # phase-1 consumption order ---
            xt_sb = [wgt.tile([128, KX * 512], F32R, tag=f"xt_{n}",
                              name=f"xt_{n}") for n in range(RG)]
            w1p_sb = [wgt.tile([128, KX * 256], F32R, tag=f"w1_{mp}",
                               name=f"w1_{mp}") for mp in range(KH // 2)]
            w2p_sb = [wgt.tile([128, 4 * Y_DIM], F32R, tag=f"w2p_{h}",
                               name=f"w2p_{h}") for h in range(2)]
            # Arrival schedule (each queue ~166GB/s, ~3us per 512KB):
            # sync:   xt0a@12 w1p1@15 w1p3@18 w2p0a@21 xt1a@24 w2p1a@27
            # scalar: xt0b@12 w1p0@12+ w1p2@18 w2p0b@21 xt1b@24 w2p1b@27
            # matching PE consumption: p1(n0) -> p2(n0) j0..3 -> p1(n1)
            # -> p2(n0) j4..7 -> p2(n1); only 16 MMs depend on the last MB.
            nc.sync.dma_start(xt_sb[0][:, 0:1024],
                              xtD[0][0].ap().bitcast(F32R))
            nc.scalar.dma_start(w1p_sb[0][:], w1D[0].ap().bitcast(F32R))
            nc.scalar.dma_start(xt_sb[0][:, 1024:2048],
                                xtD[0][1].ap().bitcast(F32R))
            nc.sync.dma_start(yrow_sb[:], yrow.ap().bitcast(F32R))
            nc.sync.dma_start(o128_sb[:], o128.ap().bitcast(F32R))
            nc.scalar.dma_start(cst_sb[:], cst.ap().bitcast(F32R))
            nc.sync.dma_start(w1p_sb[1][:], w1D[1].ap().bitcast(F32R))
            nc.scalar.dma_start(w1p_sb[2][:], w1D[2].ap().bitcast(F32R))
            nc.sync.dma_start(w1p_sb[3][:], w1D[3].ap().bitcast(F32R))
            nc.sync.dma_start(w2p_sb[0][:, 0:1024],
                              w2D[0].ap()[:, 0:1024].bitcast(F32R))
            nc.scalar.dma_start(w2p_sb[0][:, 1024:2048],
                                w2D[0].ap()[:, 1024:2048].bitcast(F32R))
            nc.sync.dma_start(xt_sb[1][:, 0:1024],
                              xtD[1][0].ap().bitcast(F32R))
            nc.scalar.dma_start(xt_sb[1][:, 1024:2048],
                                xtD[1][1].ap().bitcast(F32R))
            nc.sync.dma_start(w2p_sb[1][:, 0:1024],
                              w2D[1].ap()[:, 0:1024].bitcast(F32R))
            nc.scalar.dma_start(w2p_sb[1][:, 1024:2048],
                                w2D[1].ap()[:, 1024:2048].bitcast(F32R))
            w2_sb = [w2p_sb[j // 4][:, (j % 4) * Y_DIM:(j % 4 + 1) * Y_DIM]
                     for j in range(KH)]

            def w1_slice(k, m):
                mp, mo = m // 2, m % 2
                return w1p_sb[mp][:, k * 256 + mo * 128:
                                  k * 256 + (mo + 1) * 128]

            def xt_slice(k, n):
                return xt_sb[n][:, k * 512:(k + 1) * 512]

            # pout accumulators reuse the yb slots (free after eqc)
            pout = {n: ps.tile([1, 512], f32, tag="yb", bufs=RG,
                               name=f"po_{n}") for n in range(RG)}

            hT = [hp.tile([128, N_LOC], F32R, tag=f"h_{j}", name=f"h_{j}")
                  for j in range(KH)]

            def phase1(n):
                for m in range(KH):
                    psum = ps.tile([128, 512], f32, tag="psum", bufs=6,
                                   name=f"p1_{n}_{m}")
                    for k in range(KX):
                        nc.tensor.matmul(
                            psum[:], w1_slice(k, m), xt_slice(k, n),
                            start=(k == 0), stop=(k == KX - 1))
                    nc.scalar.activation(
                        hT[m][:, n * 512:(n + 1) * 512], psum[:],
                        AF.Relu, bias=b1_sb[:, m:m + 1])

            def p2_mms(psum_l, n, q, j0, j1):
                for j in range(j0, j1):
                    nc.tensor.matmul(
                        psum_l[:],
                        w2_sb[j][:, q * 128:(q + 1) * 128],
                        hT[j][:, n * 512:(n + 1) * 512],
                        start=(j == 0), stop=(j == KH - 1))

            pending = []  # delay ones-MMs so PE never waits on DVE prod

            def finish_group(psum_l, n, q):
                prod = prp.tile([128, 512], F32R, name=f"prod_{n}_{q}")
                nc.vector.tensor_tensor(
                    prod[:], psum_l[:], eqc_sb[(n, q)][:], OP.mult)
                pending.append((n, q, prod))

            def flush_one():
                n, q, prod = pending.pop(0)
                nc.tensor.matmul(
                    pout[n][:], ones_sb, prod[:],
                    start=(q == 0), stop=(q == QY - 1))

            # pipelined schedule matched to DMA arrivals.
            # First: 8 K=128 warmup matmuls on a self-produced tile with
            # ZERO DMA dependencies (memset -> DVE round to f32r), so they
            # run during the otherwise-dead preamble window and the PE is
            # already at the warm 2.4GHz clock when the first real
            # operands land.
            # warmup source produced on-chip (memset -> DVE round to
            # f32r): no DMA dependency, so the warmup matmuls run during
            # the otherwise-dead preamble/DMA-lead-in window
            wu_f32 = wgt.tile([128, 512], f32, tag="wuf")
            nc.gpsimd.memset(wu_f32[:], 1.0)
            wu_src = wgt.tile([128, 512], F32R, tag="wur")
            nc.vector.tensor_copy(wu_src[:], wu_f32[:])
            wu = ps.tile([128, 512], f32, tag="psum", bufs=6, name="wu")
            for _ in range(8):
                nc.tensor.matmul(wu[:], wu_src[:, 0:128],
                                 wu_src[:], start=True, stop=True)
            phase1(0)
            # ybc broadcast: K=1 matmul replicates yrow across partitions;
            # eqc masks read it straight from PSUM (needed at finish_group)
            eqc_sb = {}
            for n in range(RG):
                yb = ps.tile([128, 512], f32, tag="yb", bufs=RG,
                             name=f"yb{n}")
                nc.tensor.matmul(
                    yb[:], o128_sb[:],
                    yrow_sb[:, n * 512:(n + 1) * 512],
                    start=True, stop=True)
                for q in range(QY):
                    e = eqp.tile([128, 512], f32, tag=f"eqc_{n}_{q}",
                                 name=f"eqc_{n}_{q}")
                    nc.vector.tensor_scalar(
                        e[:], yb[:], iot_sb[:, q:q + 1], cnc_sb[:, q:q + 1],
                        OP.is_equal, OP.subtract)
                    eqc_sb[(n, q)] = e
            pl_n0 = [ps.tile([128, 512], f32, tag="psum", bufs=6,
                             name=f"pl_0_{q}") for q in range(QY)]
            for q in range(QY):           # needs only w2p0 (j0..3)
                p2_mms(pl_n0[q], 0, q, 0, KH // 2)
            phase1(1)                     # needs xt1; w2p1 streams behind
            for q in range(QY):           # finish n0 with w2p1 (j4..7)
                p2_mms(pl_n0[q], 0, q, KH // 2, KH)
                finish_group(pl_n0[q], 0, q)
            for q in range(QY):
                psum_l = ps.tile([128, 512], f32, tag="psum", bufs=6,
                                 name=f"pl_1_{q}")
                p2_mms(psum_l, 1, q, 0, KH)
                finish_group(psum_l, 1, q)
                flush_one()
            while pending:
                flush_one()

            # --- epilogue: add g, store (single output DMA) ---
            o = osb.tile([1, N_LOC], f32, tag="o")
            for n in range(RG):
                nc.vector.tensor_tensor(
                    o[:, n * 512:(n + 1) * 512], pout[n][:],
                    g_sb[:, n * 512:(n + 1) * 512], OP.add)
            nc.sync.dma_start(out.ap(), o[:])

    nc.compile()
    return nc


def _get_nc():
    if "nc" not in _NC_CACHE:
        import concourse.bacc as bacc
        import concourse.mybir as mybir
        from concourse import tile
        _NC_CACHE["nc"] = _build(bacc.Bacc, mybir, tile)
    return _NC_CACHE["nc"]


def kernel(x_samples, y_idx, W1, b1, W2, b2):
    from concourse.bass_utils import run_bass_kernel_spmd

    x = np.ascontiguousarray(np.asarray(x_samples, dtype=np.float32))
    y = np.asarray(y_idx).astype(np.int64).reshape(-1)
    W1 = np.ascontiguousarray(np.asarray(W1, dtype=np.float32))
    b1 = np.asarray(b1, dtype=np.float32).reshape(-1)
    W2 = np.ascontiguousarray(np.asarray(W2, dtype=np.float32))
    b2 = np.asarray(b2, dtype=np.float32).reshape(-1)

    # global label histogram + fully-folded bias term
    c = np.bincount(y, minlength=Y_DIM).astype(np.float32)
    cN = c / np.float32(N)
    beta = np.float32(b2 @ c) / np.float32(N)
    g_full = (b2[y] - beta).astype(np.float32)

    # device layouts: every DMA is one contiguous descriptor
    # w1_dev[mp][p, k*256+c] = W1[k*128+p, mp*256+c]
    w1_dev = np.ascontiguousarray(
        W1.reshape(KX, 128, KH // 2, 256).transpose(2, 1, 0, 3)
        .reshape(KH // 2, 128, KX * 256))
    # w2_dev[h][p, a*512+y] = W2[(h*4+a)*128+p, y]
    w2_dev = np.ascontiguousarray(
        W2.reshape(2, 4, 128, Y_DIM).transpose(0, 2, 1, 3)
        .reshape(2, 128, 4 * Y_DIM))
    b1c = b1.reshape(KH, 128).T                                   # [128, 8]
    iot = np.arange(Y_DIM, dtype=np.float32).reshape(QY, 128).T   # [128, 4]
    cNc = cN.reshape(QY, 128).T                                   # [128, 4]
    onesv = np.ones((128, 1), dtype=np.float32)
    cst = np.ascontiguousarray(
        np.concatenate([b1c, iot, cNc, onesv], axis=1))           # [128, 17]
    o128 = np.ones((1, 128), dtype=np.float32)

    in_maps = []
    for m in range(N_CORES):
        sl = slice(m * N_LOC, (m + 1) * N_LOC)
        # xt_dev[n][p, k*512+r] = x[m*N_LOC + n*512+r, k*128+p]
        xt_dev = np.ascontiguousarray(
            x[sl].reshape(RG, 512, KX, 128).transpose(0, 3, 2, 1)
            .reshape(RG, 128, KX * 512))
        im = {
            **{f"w1p{mp}": w1_dev[mp] for mp in range(KH // 2)},
            **{f"w2p{h}": w2_dev[h] for h in range(2)},
            "cst": cst,
            "o128": o128,
            "yrow": np.ascontiguousarray(
                y[sl].astype(np.float32)).reshape(1, N_LOC),
            "gv": np.ascontiguousarray(g_full[sl]).reshape(1, N_LOC),
        }
        for n in range(RG):
            im[f"xt{n}a"] = np.ascontiguousarray(xt_dev[n][:, 0:1024])
            im[f"xt{n}b"] = np.ascontiguousarray(xt_dev[n][:, 1024:2048])
        in_maps.append(im)

    nc = _get_nc()
    res = run_bass_kernel_spmd(nc, in_maps, core_ids=list(range(N_CORES)))
    return np.concatenate(
        [res.results[m]["out"].reshape(-1) for m in range(N_CORES)]
    ).astype(np.float32)



# revision 3
# speedup vs baseline: 1.2814x; 1.2814x over previous
"""Trainium2 Bass kernel for nn_CLUBCategorical (CLUB categorical loss).

Reference computation:
    h      = relu(x @ W1 + b1)              [N, H]
    logits = h @ W2 + b2                    [N, Y]
    logp   = log_softmax(logits, -1)        [N, Y]
    out[i] = logp[i, y_i] - mean_j logp[i, y_j]

The log-softmax normalizer cancels between the positive and negative
terms. With c[y] = histogram(y_idx) (global), w2c = (W2 @ c)/N:

    out[i] = h[i,:] @ (W2[:, y_i] - w2c) + (b2[y_i] - (b2 @ c)/N)
           = h[i,:] @ A[:, i] + g[i]

A is gathered on the HOST (it knows y), so the device only computes
phase-1 (h = relu(x@W1+b1)) plus a cheap fused product-reduce:

    per m-chunk (128 hidden dims):  prod_m = hT_m * A_m      (DVE, bf16)
    acc = sum_m prod_m                                        (DVE chain)
    out_rgroup[1, 512] = ones[128,1]^T @ acc                  (1 matmul)

g is added on the host during unsharding. Device PE work is just the
64 phase-1 matmuls (f32r) + 2 ones-reduce matmuls + warmup spins that
ride the DVFS ramp while the first DMAs land.

Sharding: data-parallel over N; each of 8 cores takes 1024 rows and the
full W1 plus its own gathered A block. No collectives.
"""

import numpy as np

N, X_DIM, Y_DIM, HIDDEN = 8192, 512, 512, 1024
N_CORES = 8
N_LOC = N // N_CORES          # 1024 rows per core
KX = X_DIM // 128             # 4  k-chunks (contraction), phase 1
KH = HIDDEN // 128            # 8  hidden chunks
RG = N_LOC // 512             # 2  row groups of 512

N_WU = 6                      # warmup matmuls (ride DVFS ramp pre-DMA)

_NC_CACHE = {}


def _build(nc_cls, mybir, tile):
    mdt = mybir.dt
    f32 = mdt.float32
    F32R = mdt.float32r
    BF16 = mdt.bfloat16
    AF = mybir.ActivationFunctionType
    OP = mybir.AluOpType

    nc = nc_cls("TRN2", target_bir_lowering=False, debug=False,
                num_devices=N_CORES)

    # --- dram tensors (per-core layouts; every DMA one contiguous tile) ---
    # xt{n}k{k}: [128, 512]  xt[p, r] = x[n*512+r, k*128+p]            f32
    xtD = [[nc.dram_tensor(f"xt{n}k{k}", [128, 512], f32,
                           kind="ExternalInput") for k in range(KX)]
           for n in range(RG)]
    # w1m{m}: [128, 512]    w1[p, k*128+c] = W1[k*128+p, m*128+c]      f32
    w1D = [nc.dram_tensor(f"w1m{m}", [128, KX * 128], f32,
                          kind="ExternalInput") for m in range(KH)]
    # am{m}: [128, 1024]    a[p, r] = W2[m*128+p, y[r]] - w2c[m*128+p] bf16
    aD = [nc.dram_tensor(f"am{m}", [128, N_LOC], BF16,
                         kind="ExternalInput") for m in range(KH)]
    # b1c: [128, 8]         b1c[p, m] = b1[m*128+p]                    f32
    b1D = nc.dram_tensor("b1c", [128, KH], f32, kind="ExternalInput")
    out = nc.dram_tensor("out", [1, N_LOC], f32, kind="ExternalOutput")

    with tile.TileContext(nc) as tc:
        with (
            tc.tile_pool(name="wgt", bufs=1) as wgt,
            tc.tile_pool(name="hp", bufs=1) as hp,
            tc.tile_pool(name="pr", bufs=1) as pr,
            tc.tile_pool(name="osb", bufs=1) as osb,
            tc.tile_pool(name="ps", bufs=1, space="PSUM") as ps,
        ):
            # --- on-chip constants (no DMA dependency) ---
            ones_f = wgt.tile([128, 1], f32, tag="onesf")
            nc.vector.memset(ones_f[:], 1.0)
            ones_sb = wgt.tile([128, 1], BF16, tag="ones")
            nc.vector.tensor_copy(ones_sb[:], ones_f[:])
            wu_f = wgt.tile([128, 256], f32, tag="wu")
            nc.vector.memset(wu_f[:], 1.0)
            wu_src = wu_f.bitcast(F32R)

            b1_sb = wgt.tile([128, KH], f32, tag="b1")

            xt_sb = [wgt.tile([128, KX * 512], F32R, tag=f"xt{n}",
                              name=f"xt{n}") for n in range(RG)]
            w1_sb = wgt.tile([128, KH * KX * 128], F32R, tag="w1")
            a_sb = [wgt.tile([128, N_LOC], BF16, tag=f"a{m}", name=f"a{m}")
                    for m in range(KH)]

            # --- DMA schedule: two HWDGE queues, issue order = FIFO order,
            # ordered by consumption deadline (phase-1 first, A behind) ---
            def w1_dst(m):
                return w1_sb[:, m * 512:(m + 1) * 512]

            nc.sync.dma_start(xt_sb[0][:, 0:512], xtD[0][0].ap().bitcast(F32R))
            nc.scalar.dma_start(w1_dst(0), w1D[0].ap().bitcast(F32R))
            nc.scalar.dma_start(b1_sb[:], b1D.ap())
            nc.scalar.dma_start(xt_sb[0][:, 512:1024],
                                xtD[0][1].ap().bitcast(F32R))
            nc.sync.dma_start(xt_sb[0][:, 1024:1536],
                              xtD[0][2].ap().bitcast(F32R))
            nc.scalar.dma_start(xt_sb[0][:, 1536:2048],
                                xtD[0][3].ap().bitcast(F32R))
            nc.sync.dma_start(w1_dst(1), w1D[1].ap().bitcast(F32R))
            nc.scalar.dma_start(w1_dst(2), w1D[2].ap().bitcast(F32R))
            nc.sync.dma_start(w1_dst(3), w1D[3].ap().bitcast(F32R))
            nc.scalar.dma_start(w1_dst(4), w1D[4].ap().bitcast(F32R))
            nc.sync.dma_start(w1_dst(5), w1D[5].ap().bitcast(F32R))
            nc.scalar.dma_start(w1_dst(6), w1D[6].ap().bitcast(F32R))
            nc.sync.dma_start(w1_dst(7), w1D[7].ap().bitcast(F32R))
            nc.scalar.dma_start(xt_sb[1][:, 0:512],
                                xtD[1][0].ap().bitcast(F32R))
            nc.sync.dma_start(xt_sb[1][:, 512:1024],
                              xtD[1][1].ap().bitcast(F32R))
            nc.scalar.dma_start(xt_sb[1][:, 1024:1536],
                                xtD[1][2].ap().bitcast(F32R))
            nc.sync.dma_start(xt_sb[1][:, 1536:2048],
                              xtD[1][3].ap().bitcast(F32R))
            for m in range(KH):
                eng = nc.scalar if m % 2 == 0 else nc.sync
                eng.dma_start(a_sb[m][:], aD[m].ap())

            # --- PE warmup: zero-dependency spins during the DMA lead-in ---
            wu = ps.tile([128, 512], f32, tag="psum", bufs=6, name="wu")
            for _ in range(N_WU):
                nc.tensor.matmul(wu[:, 0:256], wu_src[:, 0:128], wu_src[:],
                                 start=True, stop=True)

            hT = [hp.tile([128, N_LOC], BF16, tag=f"h{m}", name=f"h{m}")
                  for m in range(KH)]
            # per-rgroup accumulator chain tiles
            prod = {}
            acc = {}

            def phase1(n, m):
                psum = ps.tile([128, 512], f32, tag="psum", bufs=6,
                               name=f"p1_{n}_{m}")
                for k in range(KX):
                    nc.tensor.matmul(
                        psum[:],
                        w1_sb[:, m * 512 + k * 128: m * 512 + (k + 1) * 128],
                        xt_sb[n][:, k * 512:(k + 1) * 512],
                        start=(k == 0), stop=(k == KX - 1))
                nc.scalar.activation(
                    hT[m][:, n * 512:(n + 1) * 512], psum[:],
                    AF.Relu, bias=b1_sb[:, m:m + 1])

            def product(n, m):
                p = pr.tile([128, 512], BF16, tag=f"pr{n}_{m}",
                            name=f"pr{n}_{m}")
                nc.vector.tensor_tensor(
                    p[:], hT[m][:, n * 512:(n + 1) * 512],
                    a_sb[m][:, n * 512:(n + 1) * 512], OP.mult)
                prod[(n, m)] = p
                if m == 0:
                    acc[n] = p
                else:
                    a2 = pr.tile([128, 512], BF16, tag=f"ac{n}_{m}",
                                 name=f"ac{n}_{m}")
                    nc.vector.tensor_tensor(a2[:], acc[n][:], p[:], OP.add)
                    acc[n] = a2

            pout = {}

            def reduce_out(n):
                po = ps.tile([1, 512], f32, tag=f"po{n}", bufs=1,
                             name=f"po{n}")
                nc.tensor.matmul(po[:], ones_sb[:], acc[n][:],
                                 start=True, stop=True)
                pout[n] = po

            # --- schedule ---
            for m in range(KH):
                phase1(0, m)
                product(0, m)
            # rgroup-0 reduce early (between rgroup-1 m-groups)
            for m in range(KH):
                phase1(1, m)
                if m == 2:
                    reduce_out(0)
                product(1, m)
            reduce_out(1)

            # --- epilogue: psum -> sbuf -> dram (host adds g) ---
            o_sb = osb.tile([1, N_LOC], f32, tag="o")
            nc.vector.tensor_copy(o_sb[:, 0:512], pout[0][:])
            nc.sync.dma_start(out.ap()[:, 0:512], o_sb[:, 0:512])
            nc.vector.tensor_copy(o_sb[:, 512:1024], pout[1][:])
            nc.scalar.dma_start(out.ap()[:, 512:1024], o_sb[:, 512:1024])

    nc.compile()
    return nc


def _get_nc():
    if "nc" not in _NC_CACHE:
        import concourse.bacc as bacc
        import concourse.mybir as mybir
        from concourse import tile
        _NC_CACHE["nc"] = _build(bacc.Bacc, mybir, tile)
    return _NC_CACHE["nc"]


def kernel(x_samples, y_idx, W1, b1, W2, b2):
    import ml_dtypes
    from concourse.bass_utils import run_bass_kernel_spmd

    bf16 = ml_dtypes.bfloat16
    x = np.ascontiguousarray(np.asarray(x_samples, dtype=np.float32))
    y = np.asarray(y_idx).astype(np.int64).reshape(-1)
    W1 = np.ascontiguousarray(np.asarray(W1, dtype=np.float32))
    b1 = np.asarray(b1, dtype=np.float32).reshape(-1)
    W2 = np.ascontiguousarray(np.asarray(W2, dtype=np.float32))
    b2 = np.asarray(b2, dtype=np.float32).reshape(-1)

    # global label histogram; fold normalizer-free negative term + bias
    c = np.bincount(y, minlength=Y_DIM).astype(np.float32)
    w2c = (W2 @ c) / np.float32(N)                                # [H]
    beta = np.float32(b2 @ c) / np.float32(N)
    g_full = (b2[y] - beta).astype(np.float32)                    # [N]

    # device layouts
    # w1_dev[m][p, k*128+c] = W1[k*128+p, m*128+c]
    w1_dev = np.ascontiguousarray(
        W1.reshape(KX, 128, KH, 128).transpose(2, 1, 0, 3)
        .reshape(KH, 128, KX * 128))
    b1c = np.ascontiguousarray(b1.reshape(KH, 128).T)             # [128, 8]
    W2m = W2 - w2c[:, None]                                       # [H, Y]

    in_maps = []
    for mcore in range(N_CORES):
        sl = slice(mcore * N_LOC, (mcore + 1) * N_LOC)
        # xt_dev[n][k][p, r] = x[base + n*512+r, k*128+p]
        xt_dev = (x[sl].reshape(RG, 512, KX, 128).transpose(0, 2, 3, 1))
        # a_dev[m][p, r] = W2m[m*128+p, y[base+r]]
        a_dev = W2m[:, y[sl]].reshape(KH, 128, N_LOC).astype(bf16)
        im = {"b1c": b1c}
        for m in range(KH):
            im[f"w1m{m}"] = w1_dev[m]
            im[f"am{m}"] = np.ascontiguousarray(a_dev[m])
        for n in range(RG):
            for k in range(KX):
                im[f"xt{n}k{k}"] = np.ascontiguousarray(xt_dev[n][k])
        in_maps.append(im)

    nc = _get_nc()
    res = run_bass_kernel_spmd(nc, in_maps, core_ids=list(range(N_CORES)))
    dev = np.concatenate(
        [res.results[mc]["out"].reshape(-1) for mc in range(N_CORES)])
    return (dev + g_full).astype(np.float32)


# revision 5
# speedup vs baseline: 1.2833x; 1.0015x over previous
"""Trainium2 Bass kernel for nn_CLUBCategorical (CLUB categorical loss).

Reference computation:
    h      = relu(x @ W1 + b1)              [N, H]
    logits = h @ W2 + b2                    [N, Y]
    logp   = log_softmax(logits, -1)        [N, Y]
    out[i] = logp[i, y_i] - mean_j logp[i, y_j]

The log-softmax normalizer cancels between the positive and negative
terms. With c[y] = histogram(y_idx) (global), w2c = (W2 @ c)/N:

    out[i] = h[i,:] @ (W2[:, y_i] - w2c) + (b2[y_i] - (b2 @ c)/N)
           = h[i,:] @ A[:, i] + g[i]

A is gathered on the HOST (it knows y), so the device only computes
phase-1 (h = relu(x@W1+b1), 64 matmuls) plus a fused product-reduce:

    per m-chunk (128 hidden dims):  prod_m = hT_m * A_m      (DVE, bf16)
    acc = sum_{m<7} prod_m                                    (DVE chain)
    out_rg[1,512] = ones^T @ acc (+) ones^T @ prod_7          (2 matmuls,
                                          PSUM-accumulated, short tail)

g is added on the host during unsharding.

Queue discipline (the v1 lesson): the Scalar queue runs the 16 ReLU
activations, so it gets NO bulk DMA configs (configs block at ring
depth 4 and would jam the activations behind the whole load stream).
Phase-1 tensors stream on the sync (SP) HWDGE queue, A rides the
gpsimd SWDGE, Vector stays pure compute.

Sharding: data-parallel over N; each of 8 cores takes 1024 rows and the
full W1 plus its own gathered A block. No collectives.
"""

import numpy as np

N, X_DIM, Y_DIM, HIDDEN = 8192, 512, 512, 1024
N_CORES = 8
N_LOC = N // N_CORES          # 1024 rows per core
KX = X_DIM // 128             # 4  k-chunks (contraction), phase 1
KH = HIDDEN // 128            # 8  hidden chunks
RG = N_LOC // 512             # 2  row groups of 512

N_WU = 6                      # warmup matmuls (ride DVFS ramp pre-DMA)

_NC_CACHE = {}


def _build(nc_cls, mybir, tile):
    mdt = mybir.dt
    f32 = mdt.float32
    F32R = mdt.float32r
    BF16 = mdt.bfloat16
    AF = mybir.ActivationFunctionType
    OP = mybir.AluOpType

    nc = nc_cls("TRN2", target_bir_lowering=False, debug=False,
                num_devices=N_CORES)

    # --- dram tensors (bf16 device layouts; one contiguous tile per DMA) ---
    # xt{n}{a,b}: [128, 1024]  xt[p, k*512+r] = x[n*512+r, (k0+k)*128+p]
    xtD = [[nc.dram_tensor(f"xt{n}{h}", [128, 1024], BF16,
                           kind="ExternalInput") for h in "ab"]
           for n in range(RG)]
    # w1 split by consumption: m0 | m1-3 | m4-7
    # layout w1[p, m*512 + k*128 + c] = W1[k*128+p, m*128+c]
    w1D = [nc.dram_tensor("w1a", [128, 512], BF16, kind="ExternalInput"),
           nc.dram_tensor("w1b", [128, 1536], BF16, kind="ExternalInput"),
           nc.dram_tensor("w1c", [128, 2048], BF16, kind="ExternalInput")]
    # ap{j}: [128, 2048]  A chunks m=2j,2j+1; a[p, r] = W2m[m*128+p, y[r]]
    aD = [nc.dram_tensor(f"ap{j}", [128, 2 * N_LOC], BF16,
                         kind="ExternalInput") for j in range(KH // 2)]
    # b1c: [128, 8]  b1c[p, m] = b1[m*128+p]
    b1D = nc.dram_tensor("b1c", [128, KH], f32, kind="ExternalInput")
    out = nc.dram_tensor("out", [1, N_LOC], f32, kind="ExternalOutput")

    with tile.TileContext(nc) as tc:
        with (
            tc.tile_pool(name="wgt", bufs=1) as wgt,
            tc.tile_pool(name="hp", bufs=1) as hp,
            tc.tile_pool(name="pr", bufs=1) as pr,
            tc.tile_pool(name="ps", bufs=1, space="PSUM") as ps,
        ):
            # --- on-chip constants (no DMA dependency) ---
            ones_f = wgt.tile([128, 1], f32, tag="onesf")
            nc.vector.memset(ones_f[:], 1.0)
            ones_sb = wgt.tile([128, 1], BF16, tag="ones")
            nc.vector.tensor_copy(ones_sb[:], ones_f[:])
            wu_f = wgt.tile([128, 256], f32, tag="wu")
            nc.vector.memset(wu_f[:], 1.0)
            wu_src = wu_f.bitcast(F32R)

            b1_sb = wgt.tile([128, KH], f32, tag="b1")
            xt_sb = [wgt.tile([128, KX * 512], BF16, tag=f"xt{n}",
                              name=f"xt{n}") for n in range(RG)]
            w1_sb = wgt.tile([128, KH * 512], BF16, tag="w1")
            a_sb = [wgt.tile([128, 2 * N_LOC], BF16, tag=f"a{j}",
                             name=f"a{j}") for j in range(KH // 2)]

            # --- DMA: sync = phase-1 stream, gpsimd = A, scalar = b1 only ---
            nc.sync.dma_start(w1_sb[:, 0:512], w1D[0].ap())
            nc.scalar.dma_start(b1_sb[:], b1D.ap())
            nc.sync.dma_start(xt_sb[0][:, 0:1024], xtD[0][0].ap())
            nc.sync.dma_start(xt_sb[0][:, 1024:2048], xtD[0][1].ap())
            nc.sync.dma_start(w1_sb[:, 512:2048], w1D[1].ap())
            nc.sync.dma_start(w1_sb[:, 2048:4096], w1D[2].ap())
            nc.sync.dma_start(xt_sb[1][:, 0:1024], xtD[1][0].ap())
            nc.sync.dma_start(xt_sb[1][:, 1024:2048], xtD[1][1].ap())
            for j in range(KH // 2):
                nc.gpsimd.dma_start(a_sb[j][:], aD[j].ap())

            # --- PE warmup: zero-dependency spins during the DMA lead-in ---
            wu = ps.tile([128, 512], f32, tag="psum", bufs=6, name="wu")
            for _ in range(N_WU):
                nc.tensor.matmul(wu[:, 0:256], wu_src[:, 0:128], wu_src[:],
                                 start=True, stop=True)

            hT = [hp.tile([128, N_LOC], BF16, tag=f"h{m}", name=f"h{m}")
                  for m in range(KH)]
            prod = {}
            acc = {}

            def a_slice(n, m):
                return a_sb[m // 2][:, (m % 2) * N_LOC + n * 512:
                                    (m % 2) * N_LOC + (n + 1) * 512]

            def phase1(n, m):
                psum = ps.tile([128, 512], f32, tag="psum", bufs=6,
                               name=f"p1_{n}_{m}")
                for k in range(KX):
                    nc.tensor.matmul(
                        psum[:],
                        w1_sb[:, m * 512 + k * 128: m * 512 + (k + 1) * 128],
                        xt_sb[n][:, k * 512:(k + 1) * 512],
                        start=(k == 0), stop=(k == KX - 1))
                nc.scalar.activation(
                    hT[m][:, n * 512:(n + 1) * 512], psum[:],
                    AF.Relu, bias=b1_sb[:, m:m + 1])

            def product(n, m):
                p = pr.tile([128, 512], BF16, tag=f"pr{n}_{m}",
                            name=f"pr{n}_{m}")
                nc.vector.tensor_tensor(
                    p[:], hT[m][:, n * 512:(n + 1) * 512],
                    a_slice(n, m), OP.mult)
                prod[(n, m)] = p
                if m == 0:
                    acc[n] = p
                elif m < KH - 1:   # m7 joins via the PSUM-accumulated matmul
                    a2 = pr.tile([128, 512], BF16, tag=f"ac{n}_{m}",
                                 name=f"ac{n}_{m}")
                    nc.vector.tensor_tensor(a2[:], acc[n][:], p[:], OP.add)
                    acc[n] = a2

            pout = {}

            def reduce_pre(n):     # ones^T @ acc(m0..m6) -> pout[n]
                po = ps.tile([1, 512], f32, tag=f"po{n}", bufs=1,
                             name=f"po{n}")
                nc.tensor.matmul(po[:], ones_sb[:], acc[n][:],
                                 start=True, stop=False)
                pout[n] = po

            o_sb = wgt.tile([1, N_LOC], f32, tag="o")

            def reduce_fin(n):     # += ones^T @ prod_7, copy out, DMA
                nc.tensor.matmul(pout[n][:], ones_sb[:], prod[(n, KH - 1)][:],
                                 start=False, stop=True)
                nc.vector.tensor_copy(o_sb[:, n * 512:(n + 1) * 512],
                                      pout[n][:])
                eng = nc.sync if n == 0 else nc.scalar
                eng.dma_start(out.ap()[:, n * 512:(n + 1) * 512],
                              o_sb[:, n * 512:(n + 1) * 512])

            # --- schedule ---
            for m in range(KH):
                phase1(0, m)
                product(0, m)
            for m in range(KH):
                phase1(1, m)
                if m == 1:
                    reduce_pre(0)
                elif m == 2:
                    reduce_fin(0)
                product(1, m)
            reduce_pre(1)
            reduce_fin(1)

    nc.compile()
    return nc


def _get_nc():
    if "nc" not in _NC_CACHE:
        import concourse.bacc as bacc
        import concourse.mybir as mybir
        from concourse import tile
        _NC_CACHE["nc"] = _build(bacc.Bacc, mybir, tile)
    return _NC_CACHE["nc"]


def kernel(x_samples, y_idx, W1, b1, W2, b2):
    import ml_dtypes
    from concourse.bass_utils import run_bass_kernel_spmd

    bf16 = ml_dtypes.bfloat16
    x = np.ascontiguousarray(np.asarray(x_samples, dtype=np.float32))
    y = np.asarray(y_idx).astype(np.int64).reshape(-1)
    W1 = np.ascontiguousarray(np.asarray(W1, dtype=np.float32))
    b1 = np.asarray(b1, dtype=np.float32).reshape(-1)
    W2 = np.ascontiguousarray(np.asarray(W2, dtype=np.float32))
    b2 = np.asarray(b2, dtype=np.float32).reshape(-1)

    # global label histogram; fold normalizer-free negative term + bias
    c = np.bincount(y, minlength=Y_DIM).astype(np.float32)
    w2c = (W2 @ c) / np.float32(N)                                # [H]
    beta = np.float32(b2 @ c) / np.float32(N)
    g_full = (b2[y] - beta).astype(np.float32)                    # [N]

    # device layouts
    # w1_dev[m][p, k*128+c] = W1[k*128+p, m*128+c]
    w1_dev = np.ascontiguousarray(
        W1.reshape(KX, 128, KH, 128).transpose(2, 1, 0, 3)
        .reshape(KH, 128, KX * 128)).astype(bf16)
    w1_flat = np.ascontiguousarray(
        w1_dev.transpose(1, 0, 2).reshape(128, KH * 512))
    b1c = np.ascontiguousarray(b1.reshape(KH, 128).T)             # [128, 8]
    W2m = W2 - w2c[:, None]                                       # [H, Y]

    in_maps = []
    for mcore in range(N_CORES):
        sl = slice(mcore * N_LOC, (mcore + 1) * N_LOC)
        # xt_dev[n][p, k*512+r] = x[base + n*512+r, k*128+p]
        xt_dev = np.ascontiguousarray(
            x[sl].reshape(RG, 512, KX, 128).transpose(0, 3, 2, 1)
            .reshape(RG, 128, KX * 512)).astype(bf16)
        # a_dev[m][p, r] = W2m[m*128+p, y[base+r]]
        a_dev = W2m[:, y[sl]].reshape(KH, 128, N_LOC).astype(bf16)
        im = {
            "b1c": b1c,
            "w1a": np.ascontiguousarray(w1_flat[:, 0:512]),
            "w1b": np.ascontiguousarray(w1_flat[:, 512:2048]),
            "w1c": np.ascontiguousarray(w1_flat[:, 2048:4096]),
        }
        for j in range(KH // 2):
            im[f"ap{j}"] = np.ascontiguousarray(
                a_dev[2 * j:2 * j + 2].transpose(1, 0, 2)
                .reshape(128, 2 * N_LOC))
        for n in range(RG):
            im[f"xt{n}a"] = np.ascontiguousarray(xt_dev[n][:, 0:1024])
            im[f"xt{n}b"] = np.ascontiguousarray(xt_dev[n][:, 1024:2048])
        in_maps.append(im)

    nc = _get_nc()
    res = run_bass_kernel_spmd(nc, in_maps, core_ids=list(range(N_CORES)))
    dev = np.concatenate(
        [res.results[mc]["out"].reshape(-1) for mc in range(N_CORES)])
    return (dev + g_full).astype(np.float32)


# revision 9
# speedup vs baseline: 1.3336x; 1.0392x over previous
"""Trainium2 Bass kernel for nn_CLUBCategorical (CLUB categorical loss).

Reference computation:
    h      = relu(x @ W1 + b1)              [N, H]
    logits = h @ W2 + b2                    [N, Y]
    logp   = log_softmax(logits, -1)        [N, Y]
    out[i] = logp[i, y_i] - mean_j logp[i, y_j]

The log-softmax normalizer cancels between the positive and negative
terms. With c[y] = histogram(y_idx) (global), w2c = (W2 @ c)/N:

    out[i] = h[i,:] @ (W2[:, y_i] - w2c) + (b2[y_i] - (b2 @ c)/N)
           = h[i,:] @ A[:, i] + g[i]

A is gathered on the HOST (it knows y), so the device only computes
phase-1 (h = relu(x@W1+b1), 64 matmuls) plus a fused product-reduce:

    per m-chunk (128 hidden dims):  prod_m = hT_m * A_m      (DVE, bf16)
    acc = sum_{m<7} prod_m                                    (DVE chain)
    out_rg[1,512] = ones^T @ acc (+) ones^T @ prod_7          (2 matmuls,
                                          PSUM-accumulated, short tail)

g is added on the host during unsharding.

Queue discipline (the v1 lesson): the Scalar queue runs the 16 ReLU
activations, so it gets NO bulk DMA configs (configs block at ring
depth 4 and would jam the activations behind the whole load stream).
Phase-1 tensors stream on the sync (SP) HWDGE queue, A rides the
gpsimd SWDGE, Vector stays pure compute.

Sharding: data-parallel over N; each of 8 cores takes 1024 rows and the
full W1 plus its own gathered A block. No collectives.
"""

import numpy as np

N, X_DIM, Y_DIM, HIDDEN = 8192, 512, 512, 1024
N_CORES = 8
N_LOC = N // N_CORES          # 1024 rows per core
KX = X_DIM // 128             # 4  k-chunks (contraction), phase 1
KH = HIDDEN // 128            # 8  hidden chunks
RG = N_LOC // 512             # 2  row groups of 512

N_WU = 5                      # warmup matmuls (ride DVFS ramp pre-DMA)

_NC_CACHE = {}


def _build(nc_cls, mybir, tile):
    mdt = mybir.dt
    f32 = mdt.float32
    F32R = mdt.float32r
    BF16 = mdt.bfloat16
    AF = mybir.ActivationFunctionType
    OP = mybir.AluOpType

    nc = nc_cls("TRN2", target_bir_lowering=False, debug=False,
                num_devices=N_CORES)

    # --- dram tensors (bf16 device layouts; one contiguous tile per DMA) ---
    # xt{n}{a,b}: [128, 1024]  xt[p, k*512+r] = x[n*512+r, (k0+k)*128+p]
    xtD = [[nc.dram_tensor(f"xt{n}{h}", [128, 1024], BF16,
                           kind="ExternalInput") for h in "ab"]
           for n in range(RG)]
    # w1 split by consumption: m0 | m1-3 | m4-7
    # layout w1[p, m*512 + k*128 + c] = W1[k*128+p, m*128+c]
    w1D = [nc.dram_tensor("w1a", [128, 512], BF16, kind="ExternalInput"),
           nc.dram_tensor("w1b", [128, 1536], BF16, kind="ExternalInput"),
           nc.dram_tensor("w1c", [128, 2048], BF16, kind="ExternalInput")]
    # ap{j}: [128, 2048]  A chunks m=2j,2j+1; a[p, r] = W2m[m*128+p, y[r]]
    aD = [nc.dram_tensor(f"ap{j}", [128, 2 * N_LOC], BF16,
                         kind="ExternalInput") for j in range(KH // 2)]
    # b1c: [128, 8]  b1c[p, m] = b1[m*128+p]
    b1D = nc.dram_tensor("b1c", [128, KH], f32, kind="ExternalInput")
    out = nc.dram_tensor("out", [1, N_LOC], f32, kind="ExternalOutput")

    with tile.TileContext(nc) as tc:
        with (
            tc.tile_pool(name="wgt", bufs=1) as wgt,
            tc.tile_pool(name="hp", bufs=1) as hp,
            tc.tile_pool(name="pr", bufs=1) as pr,
            tc.tile_pool(name="ps", bufs=1, space="PSUM") as ps,
        ):
            # --- on-chip constants (no DMA dependency; wu first so the
            # PE warmup spins can start as early as possible) ---
            wu_f = wgt.tile([128, 512], f32, tag="wu")
            nc.vector.memset(wu_f[:], 1.0)
            wu_src = wu_f.bitcast(F32R)
            ones_f = wgt.tile([128, 1], f32, tag="onesf")
            nc.vector.memset(ones_f[:], 1.0)
            ones_sb = wgt.tile([128, 1], BF16, tag="ones")
            nc.vector.tensor_copy(ones_sb[:], ones_f[:])

            b1_sb = wgt.tile([128, KH], f32, tag="b1")
            xt_sb = [wgt.tile([128, KX * 512], BF16, tag=f"xt{n}",
                              name=f"xt{n}") for n in range(RG)]
            w1_sb = wgt.tile([128, KH * 512], BF16, tag="w1")
            a_sb = [wgt.tile([128, 2 * N_LOC], BF16, tag=f"a{j}",
                             name=f"a{j}") for j in range(KH // 2)]

            # --- DMA: sync = W1 + x1 + A67, scalar = b1 + x0 (its two
            # configs clear before the ReLUs), gpsimd SWDGE = A01..A45 ---
            nc.sync.dma_start(w1_sb[:, 0:512], w1D[0].ap())
            nc.scalar.dma_start(b1_sb[:], b1D.ap())
            nc.scalar.dma_start(xt_sb[0][:, 0:1024], xtD[0][0].ap())
            nc.scalar.dma_start(xt_sb[0][:, 1024:2048], xtD[0][1].ap())
            nc.sync.dma_start(w1_sb[:, 512:2048], w1D[1].ap())
            nc.sync.dma_start(w1_sb[:, 2048:4096], w1D[2].ap())
            nc.sync.dma_start(xt_sb[1][:, 0:1024], xtD[1][0].ap())
            nc.sync.dma_start(xt_sb[1][:, 1024:2048], xtD[1][1].ap())
            for j in range(KH // 2 - 1):
                nc.gpsimd.dma_start(a_sb[j][:], aD[j].ap())
            nc.sync.dma_start(a_sb[3][:], aD[3].ap())

            # --- PE warmup: zero-dependency spins during the DMA lead-in ---
            wu = ps.tile([128, 512], f32, tag="psum", bufs=6, name="wu")
            for _ in range(N_WU):
                nc.tensor.matmul(wu[:], wu_src[:, 0:128], wu_src[:],
                                 start=True, stop=True)

            hT = [hp.tile([128, N_LOC], BF16, tag=f"h{m}", name=f"h{m}")
                  for m in range(KH)]
            prod = {}
            acc = {}

            def a_slice(n, m):
                return a_sb[m // 2][:, (m % 2) * N_LOC + n * 512:
                                    (m % 2) * N_LOC + (n + 1) * 512]

            def phase1(n, m):
                psum = ps.tile([128, 512], f32, tag="psum", bufs=6,
                               name=f"p1_{n}_{m}")
                for k in range(KX):
                    nc.tensor.matmul(
                        psum[:],
                        w1_sb[:, m * 512 + k * 128: m * 512 + (k + 1) * 128],
                        xt_sb[n][:, k * 512:(k + 1) * 512],
                        start=(k == 0), stop=(k == KX - 1))
                nc.scalar.activation(
                    hT[m][:, n * 512:(n + 1) * 512], psum[:],
                    AF.Relu, bias=b1_sb[:, m:m + 1])

            def product(n, m):
                p = pr.tile([128, 512], BF16, tag=f"pr{n}_{m}",
                            name=f"pr{n}_{m}")
                nc.vector.tensor_tensor(
                    p[:], hT[m][:, n * 512:(n + 1) * 512],
                    a_slice(n, m), OP.mult)
                prod[(n, m)] = p
                if m == 0:
                    acc[n] = p
                elif m < KH - 1:   # m7 joins via the PSUM-accumulated matmul
                    a2 = pr.tile([128, 512], BF16, tag=f"ac{n}_{m}",
                                 name=f"ac{n}_{m}")
                    nc.vector.tensor_tensor(a2[:], acc[n][:], p[:], OP.add)
                    acc[n] = a2

            pout = {}

            def reduce_pre(n):     # ones^T @ acc(m0..m6) -> pout[n]
                po = ps.tile([1, 512], f32, tag=f"po{n}", bufs=1,
                             name=f"po{n}")
                nc.tensor.matmul(po[:], ones_sb[:], acc[n][:],
                                 start=True, stop=False)
                pout[n] = po

            o_sb = wgt.tile([1, N_LOC], f32, tag="o")

            def reduce_fin(n):     # += ones^T @ prod_7, copy out, DMA
                nc.tensor.matmul(pout[n][:], ones_sb[:], prod[(n, KH - 1)][:],
                                 start=False, stop=True)
                nc.vector.tensor_copy(o_sb[:, n * 512:(n + 1) * 512],
                                      pout[n][:])
                eng = nc.sync if n == 0 else nc.scalar
                eng.dma_start(out.ap()[:, n * 512:(n + 1) * 512],
                              o_sb[:, n * 512:(n + 1) * 512])

            # --- schedule ---
            for m in range(KH):
                phase1(0, m)
                product(0, m)
            for m in range(KH):
                phase1(1, m)
                if m == 1:
                    reduce_pre(0)
                elif m == 2:
                    reduce_fin(0)
                product(1, m)
            reduce_pre(1)
            reduce_fin(1)

    nc.compile()
    return nc


def _get_nc():
    if "nc" not in _NC_CACHE:
        import concourse.bacc as bacc
        import concourse.mybir as mybir
        from concourse import tile
        _NC_CACHE["nc"] = _build(bacc.Bacc, mybir, tile)
    return _NC_CACHE["nc"]


def kernel(x_samples, y_idx, W1, b1, W2, b2):
    import ml_dtypes
    from concourse.bass_utils import run_bass_kernel_spmd

    bf16 = ml_dtypes.bfloat16
    x = np.ascontiguousarray(np.asarray(x_samples, dtype=np.float32))
    y = np.asarray(y_idx).astype(np.int64).reshape(-1)
    W1 = np.ascontiguousarray(np.asarray(W1, dtype=np.float32))
    b1 = np.asarray(b1, dtype=np.float32).reshape(-1)
    W2 = np.ascontiguousarray(np.asarray(W2, dtype=np.float32))
    b2 = np.asarray(b2, dtype=np.float32).reshape(-1)

    # global label histogram; fold normalizer-free negative term + bias
    c = np.bincount(y, minlength=Y_DIM).astype(np.float32)
    w2c = (W2 @ c) / np.float32(N)                                # [H]
    beta = np.float32(b2 @ c) / np.float32(N)
    g_full = (b2[y] - beta).astype(np.float32)                    # [N]

    # device layouts
    # w1_dev[m][p, k*128+c] = W1[k*128+p, m*128+c]
    w1_dev = np.ascontiguousarray(
        W1.reshape(KX, 128, KH, 128).transpose(2, 1, 0, 3)
        .reshape(KH, 128, KX * 128)).astype(bf16)
    w1_flat = np.ascontiguousarray(
        w1_dev.transpose(1, 0, 2).reshape(128, KH * 512))
    b1c = np.ascontiguousarray(b1.reshape(KH, 128).T)             # [128, 8]
    W2m = W2 - w2c[:, None]                                       # [H, Y]

    in_maps = []
    for mcore in range(N_CORES):
        sl = slice(mcore * N_LOC, (mcore + 1) * N_LOC)
        # xt_dev[n][p, k*512+r] = x[base + n*512+r, k*128+p]
        xt_dev = np.ascontiguousarray(
            x[sl].reshape(RG, 512, KX, 128).transpose(0, 3, 2, 1)
            .reshape(RG, 128, KX * 512)).astype(bf16)
        # a_dev[m][p, r] = W2m[m*128+p, y[base+r]]
        a_dev = W2m[:, y[sl]].reshape(KH, 128, N_LOC).astype(bf16)
        im = {
            "b1c": b1c,
            "w1a": np.ascontiguousarray(w1_flat[:, 0:512]),
            "w1b": np.ascontiguousarray(w1_flat[:, 512:2048]),
            "w1c": np.ascontiguousarray(w1_flat[:, 2048:4096]),
        }
        for j in range(KH // 2):
            im[f"ap{j}"] = np.ascontiguousarray(
                a_dev[2 * j:2 * j + 2].transpose(1, 0, 2)
                .reshape(128, 2 * N_LOC))
        for n in range(RG):
            im[f"xt{n}a"] = np.ascontiguousarray(xt_dev[n][:, 0:1024])
            im[f"xt{n}b"] = np.ascontiguousarray(xt_dev[n][:, 1024:2048])
        in_maps.append(im)

    nc = _get_nc()
    res = run_bass_kernel_spmd(nc, in_maps, core_ids=list(range(N_CORES)))
    dev = np.concatenate(
        [res.results[mc]["out"].reshape(-1) for mc in range(N_CORES)])
    return (dev + g_full).astype(np.float32)


# revision 10
# speedup vs baseline: 1.3511x; 1.0131x over previous
"""Trainium2 Bass kernel for nn_CLUBCategorical (CLUB categorical loss).

Reference computation:
    h      = relu(x @ W1 + b1)              [N, H]
    logits = h @ W2 + b2                    [N, Y]
    logp   = log_softmax(logits, -1)        [N, Y]
    out[i] = logp[i, y_i] - mean_j logp[i, y_j]

The log-softmax normalizer cancels between the positive and negative
terms. With c[y] = histogram(y_idx) (global), w2c = (W2 @ c)/N:

    out[i] = h[i,:] @ (W2[:, y_i] - w2c) + (b2[y_i] - (b2 @ c)/N)
           = h[i,:] @ A[:, i] + g[i]

A is gathered on the HOST (it knows y), so the device only computes
phase-1 (h = relu(x@W1+b1), 64 matmuls) plus a fused product-reduce:

    per m-chunk (128 hidden dims):  prod_m = hT_m * A_m      (DVE, bf16)
    acc = sum_{m<7} prod_m                                    (DVE chain)
    out_rg[1,512] = ones^T @ acc (+) ones^T @ prod_7          (2 matmuls,
                                          PSUM-accumulated, short tail)

g is added on the host during unsharding.

Queue discipline (the v1 lesson): the Scalar queue runs the 16 ReLU
activations, so it gets NO bulk DMA configs (configs block at ring
depth 4 and would jam the activations behind the whole load stream).
Phase-1 tensors stream on the sync (SP) HWDGE queue, A rides the
gpsimd SWDGE, Vector stays pure compute.

Sharding: data-parallel over N; each of 8 cores takes 1024 rows and the
full W1 plus its own gathered A block. No collectives.
"""

import numpy as np

N, X_DIM, Y_DIM, HIDDEN = 8192, 512, 512, 1024
N_CORES = 8
N_LOC = N // N_CORES          # 1024 rows per core
KX = X_DIM // 128             # 4  k-chunks (contraction), phase 1
KH = HIDDEN // 128            # 8  hidden chunks
RG = N_LOC // 512             # 2  row groups of 512

N_WU = 5                      # warmup matmuls (ride DVFS ramp pre-DMA)

_NC_CACHE = {}


def _build(nc_cls, mybir, tile):
    mdt = mybir.dt
    f32 = mdt.float32
    F32R = mdt.float32r
    BF16 = mdt.bfloat16
    AF = mybir.ActivationFunctionType
    OP = mybir.AluOpType

    nc = nc_cls("TRN2", target_bir_lowering=False, debug=False,
                num_devices=N_CORES)

    # --- dram tensors (bf16 device layouts; one contiguous tile per DMA) ---
    # xt{n}{a,b}: [128, 1024]  xt[p, k*512+r] = x[n*512+r, (k0+k)*128+p]
    xtD = [[nc.dram_tensor(f"xt{n}{h}", [128, 1024], BF16,
                           kind="ExternalInput") for h in "ab"]
           for n in range(RG)]
    # w1 split by consumption: m0 | m1-3 | m4-7
    # layout w1[p, m*512 + k*128 + c] = W1[k*128+p, m*128+c]
    w1D = [nc.dram_tensor("w1a", [128, 512], BF16, kind="ExternalInput"),
           nc.dram_tensor("w1b", [128, 1536], BF16, kind="ExternalInput"),
           nc.dram_tensor("w1c", [128, 2048], BF16, kind="ExternalInput")]
    # ap{j}: [128, 2048]  A chunks m=2j,2j+1; a[p, r] = W2m[m*128+p, y[r]]
    aD = [nc.dram_tensor(f"ap{j}", [128, 2 * N_LOC], BF16,
                         kind="ExternalInput") for j in range(KH // 2)]
    # b1c: [128, 8]  b1c[p, m] = b1[m*128+p]
    b1D = nc.dram_tensor("b1c", [128, KH], f32, kind="ExternalInput")
    out = nc.dram_tensor("out", [1, N_LOC], f32, kind="ExternalOutput")

    with tile.TileContext(nc) as tc:
        with (
            tc.tile_pool(name="wgt", bufs=1) as wgt,
            tc.tile_pool(name="hp", bufs=1) as hp,
            tc.tile_pool(name="pr", bufs=1) as pr,
            tc.tile_pool(name="ps", bufs=1, space="PSUM") as ps,
        ):
            # --- on-chip constants (no DMA dependency; wu first so the
            # PE warmup spins can start as early as possible) ---
            wu_f = wgt.tile([128, 512], f32, tag="wu")
            nc.vector.memset(wu_f[:], 1.0)
            wu_src = wu_f.bitcast(F32R)
            ones_f = wgt.tile([128, 1], f32, tag="onesf")
            nc.vector.memset(ones_f[:], 1.0)
            ones_sb = wgt.tile([128, 1], BF16, tag="ones")
            nc.vector.tensor_copy(ones_sb[:], ones_f[:])

            b1_sb = wgt.tile([128, KH], f32, tag="b1")
            xt_sb = [wgt.tile([128, KX * 512], BF16, tag=f"xt{n}",
                              name=f"xt{n}") for n in range(RG)]
            w1_sb = wgt.tile([128, KH * 512], BF16, tag="w1")
            a_sb = [wgt.tile([128, 2 * N_LOC], BF16, tag=f"a{j}",
                             name=f"a{j}") for j in range(KH // 2)]

            # --- DMA: sync = W1 + x1 + A67, scalar = x0 + b1 (its three
            # configs clear before the ReLUs), gpsimd SWDGE = A01..A45.
            # The gpsimd configs are gated on xt0b's arrival so the A
            # stream never competes with the critical phase-1 prefix. ---
            nc.sync.dma_start(w1_sb[:, 0:512], w1D[0].ap())
            nc.scalar.dma_start(xt_sb[0][:, 0:1024], xtD[0][0].ap())
            nc.scalar.dma_start(xt_sb[0][:, 1024:2048], xtD[0][1].ap())
            nc.scalar.dma_start(b1_sb[:], b1D.ap())
            nc.sync.dma_start(w1_sb[:, 512:2048], w1D[1].ap())
            nc.sync.dma_start(w1_sb[:, 2048:4096], w1D[2].ap())
            nc.sync.dma_start(xt_sb[1][:, 0:1024], xtD[1][0].ap())
            nc.sync.dma_start(xt_sb[1][:, 1024:2048], xtD[1][1].ap())
            trig = wgt.tile([1, 1], BF16, tag="trig")
            nc.gpsimd.tensor_scalar_add(trig[:], xt_sb[0][0:1, 2047:2048],
                                        0.0)
            for j in range(KH // 2 - 1):
                nc.gpsimd.dma_start(a_sb[j][:], aD[j].ap())
            nc.sync.dma_start(a_sb[3][:], aD[3].ap())

            # --- PE warmup: zero-dependency spins during the DMA lead-in ---
            wu = ps.tile([128, 512], f32, tag="psum", bufs=6, name="wu")
            for _ in range(N_WU):
                nc.tensor.matmul(wu[:], wu_src[:, 0:128], wu_src[:],
                                 start=True, stop=True)

            hT = [hp.tile([128, N_LOC], BF16, tag=f"h{m}", name=f"h{m}")
                  for m in range(KH)]
            prod = {}
            acc = {}

            def a_slice(n, m):
                return a_sb[m // 2][:, (m % 2) * N_LOC + n * 512:
                                    (m % 2) * N_LOC + (n + 1) * 512]

            def phase1(n, m):
                psum = ps.tile([128, 512], f32, tag="psum", bufs=6,
                               name=f"p1_{n}_{m}")
                for k in range(KX):
                    nc.tensor.matmul(
                        psum[:],
                        w1_sb[:, m * 512 + k * 128: m * 512 + (k + 1) * 128],
                        xt_sb[n][:, k * 512:(k + 1) * 512],
                        start=(k == 0), stop=(k == KX - 1))
                nc.scalar.activation(
                    hT[m][:, n * 512:(n + 1) * 512], psum[:],
                    AF.Relu, bias=b1_sb[:, m:m + 1])

            def product(n, m):
                p = pr.tile([128, 512], BF16, tag=f"pr{n}_{m}",
                            name=f"pr{n}_{m}")
                nc.vector.tensor_tensor(
                    p[:], hT[m][:, n * 512:(n + 1) * 512],
                    a_slice(n, m), OP.mult)
                prod[(n, m)] = p
                if m == 0:
                    acc[n] = p
                elif m < KH - 1:   # m7 joins via the PSUM-accumulated matmul
                    a2 = pr.tile([128, 512], BF16, tag=f"ac{n}_{m}",
                                 name=f"ac{n}_{m}")
                    nc.vector.tensor_tensor(a2[:], acc[n][:], p[:], OP.add)
                    acc[n] = a2

            pout = {}

            def reduce_pre(n):     # ones^T @ acc(m0..m6) -> pout[n]
                po = ps.tile([1, 512], f32, tag=f"po{n}", bufs=1,
                             name=f"po{n}")
                nc.tensor.matmul(po[:], ones_sb[:], acc[n][:],
                                 start=True, stop=False)
                pout[n] = po

            o_sb = wgt.tile([1, N_LOC], f32, tag="o")

            def reduce_fin(n):     # += ones^T @ prod_7, copy out, DMA
                nc.tensor.matmul(pout[n][:], ones_sb[:], prod[(n, KH - 1)][:],
                                 start=False, stop=True)
                nc.vector.tensor_copy(o_sb[:, n * 512:(n + 1) * 512],
                                      pout[n][:])
                eng = nc.sync if n == 0 else nc.scalar
                eng.dma_start(out.ap()[:, n * 512:(n + 1) * 512],
                              o_sb[:, n * 512:(n + 1) * 512])

            # --- schedule ---
            for m in range(KH):
                phase1(0, m)
                product(0, m)
            for m in range(KH):
                phase1(1, m)
                if m == 1:
                    reduce_pre(0)
                elif m == 2:
                    reduce_fin(0)
                product(1, m)
            reduce_pre(1)
            reduce_fin(1)

    nc.compile()
    return nc


def _get_nc():
    if "nc" not in _NC_CACHE:
        import concourse.bacc as bacc
        import concourse.mybir as mybir
        from concourse import tile
        _NC_CACHE["nc"] = _build(bacc.Bacc, mybir, tile)
    return _NC_CACHE["nc"]


def kernel(x_samples, y_idx, W1, b1, W2, b2):
    import ml_dtypes
    from concourse.bass_utils import run_bass_kernel_spmd

    bf16 = ml_dtypes.bfloat16
    x = np.ascontiguousarray(np.asarray(x_samples, dtype=np.float32))
    y = np.asarray(y_idx).astype(np.int64).reshape(-1)
    W1 = np.ascontiguousarray(np.asarray(W1, dtype=np.float32))
    b1 = np.asarray(b1, dtype=np.float32).reshape(-1)
    W2 = np.ascontiguousarray(np.asarray(W2, dtype=np.float32))
    b2 = np.asarray(b2, dtype=np.float32).reshape(-1)

    # global label histogram; fold normalizer-free negative term + bias
    c = np.bincount(y, minlength=Y_DIM).astype(np.float32)
    w2c = (W2 @ c) / np.float32(N)                                # [H]
    beta = np.float32(b2 @ c) / np.float32(N)
    g_full = (b2[y] - beta).astype(np.float32)                    # [N]

    # device layouts
    # w1_dev[m][p, k*128+c] = W1[k*128+p, m*128+c]
    w1_dev = np.ascontiguousarray(
        W1.reshape(KX, 128, KH, 128).transpose(2, 1, 0, 3)
        .reshape(KH, 128, KX * 128)).astype(bf16)
    w1_flat = np.ascontiguousarray(
        w1_dev.transpose(1, 0, 2).reshape(128, KH * 512))
    b1c = np.ascontiguousarray(b1.reshape(KH, 128).T)             # [128, 8]
    W2m = W2 - w2c[:, None]                                       # [H, Y]

    in_maps = []
    for mcore in range(N_CORES):
        sl = slice(mcore * N_LOC, (mcore + 1) * N_LOC)
        # xt_dev[n][p, k*512+r] = x[base + n*512+r, k*128+p]
        xt_dev = np.ascontiguousarray(
            x[sl].reshape(RG, 512, KX, 128).transpose(0, 3, 2, 1)
            .reshape(RG, 128, KX * 512)).astype(bf16)
        # a_dev[m][p, r] = W2m[m*128+p, y[base+r]]
        a_dev = W2m[:, y[sl]].reshape(KH, 128, N_LOC).astype(bf16)
        im = {
            "b1c": b1c,
            "w1a": np.ascontiguousarray(w1_flat[:, 0:512]),
            "w1b": np.ascontiguousarray(w1_flat[:, 512:2048]),
            "w1c": np.ascontiguousarray(w1_flat[:, 2048:4096]),
        }
        for j in range(KH // 2):
            im[f"ap{j}"] = np.ascontiguousarray(
                a_dev[2 * j:2 * j + 2].transpose(1, 0, 2)
                .reshape(128, 2 * N_LOC))
        for n in range(RG):
            im[f"xt{n}a"] = np.ascontiguousarray(xt_dev[n][:, 0:1024])
            im[f"xt{n}b"] = np.ascontiguousarray(xt_dev[n][:, 1024:2048])
        in_maps.append(im)

    nc = _get_nc()
    res = run_bass_kernel_spmd(nc, in_maps, core_ids=list(range(N_CORES)))
    dev = np.concatenate(
        [res.results[mc]["out"].reshape(-1) for mc in range(N_CORES)])
    return (dev + g_full).astype(np.float32)
